# revision 5
# baseline (speedup 1.0000x reference)
"""Trainium2 Bass kernel for nn_Always (sliding-window smoothed-min).

The reference "scan" is a sliding-window reduction:
    out[b, t, d] = -(1/5) * log( sum_{k=0..15} exp(-5 * x[b, t-k, d]) )
with x[b, j, d] := x[b, 0, d] for j < 0 (the h0 padding).

Strategy (pure data parallel over 8 cores; 2 batches x 2 tensors per core):
  - layout: time tiles of 128 timesteps on SBUF partitions, 64 d-columns free
  - ScalarE: E = exp(-5x)
  - TensorE: banded-matrix matmuls compute the 16-wide window sum S
      (W_in: within-tile band; W_halo: previous-tile band; W_first: t=0 pad)
  - ScalarE: ln(S) from PSUM
  - VectorE: * -1/5
"""

import numpy as np

B, T, D = 16, 8192, 64
N_CORES = 8
B_PER_CORE = B // N_CORES  # 2
SCALE = 5.0
WIN = 16
P = 128                     # timesteps per time-tile (SBUF partitions)
TILE_D = D                  # free columns per time-tile
CHUNK_TILES = 32            # time tiles per chunk
CHUNK_COLS = CHUNK_TILES * TILE_D          # 2048
CHUNKS_PER_SEQ = T // (P * CHUNK_TILES)    # 2
GROUP_COLS = 512            # one PSUM bank of fp32
GROUPS = CHUNK_COLS // GROUP_COLS          # 4


def _weight_mats():
    t = np.arange(P)
    diff = t[None, :] - t[:, None]             # [t', t] -> t - t'
    w_in = ((diff >= 0) & (diff <= WIN - 1)).astype(np.float32)
    diff2 = (t[None, :] + P) - t[:, None]
    w_halo = ((diff2 >= 1) & (diff2 <= WIN - 1)).astype(np.float32)
    w_first = np.zeros((P, P), np.float32)
    w_first[0, :] = np.maximum(WIN - 1 - t, 0)
    return w_in, w_halo, w_first


def _build_bass():
    from contextlib import ExitStack

    import concourse.bass as bass
    import concourse.bacc as bacc
    import concourse.tile as tile
    from concourse import mybir

    f32 = mybir.dt.float32
    f32r = mybir.dt.float32r
    AF = mybir.ActivationFunctionType

    nc = bacc.Bacc(trn_type="TRN2")
    lo = nc.dram_tensor("lower", [B_PER_CORE, T, D], f32, kind="ExternalInput")
    up = nc.dram_tensor("upper", [B_PER_CORE, T, D], f32, kind="ExternalInput")
    out_lo = nc.dram_tensor("out_lower", [B_PER_CORE, T, D], f32, kind="ExternalOutput")
    out_up = nc.dram_tensor("out_upper", [B_PER_CORE, T, D], f32, kind="ExternalOutput")

    w_all_np = np.concatenate(_weight_mats(), axis=1)   # [128, 384]
    w_all_d = nc.inline_tensor(w_all_np, name="w_all_c")

    with tile.TileContext(nc) as tc, ExitStack() as ctx:
        consts = ctx.enter_context(tc.tile_pool(name="consts", bufs=1))
        x_pool = ctx.enter_context(tc.tile_pool(name="x", bufs=3))
        e_pool = ctx.enter_context(tc.tile_pool(name="e", bufs=3))
        o_pool = ctx.enter_context(tc.tile_pool(name="o", bufs=3))
        ps_pool = ctx.enter_context(tc.tile_pool(name="ps", bufs=2, space="PSUM"))

        w_all = consts.tile([P, 3 * P], f32r)
        nc.sync.dma_start(w_all[:], w_all_d[:].bitcast(f32r))
        w_in_r = w_all[:, 0:P]
        w_halo_r = w_all[:, P : 2 * P]
        w_first_r = w_all[:, 2 * P : 3 * P]

        seqs = []
        for src, dst in ((lo, out_lo), (up, out_up)):
            for b in range(B_PER_CORE):
                seqs.append((src[b], dst[b]))

        for x, y in seqs:
            xv = x.rearrange("(j p) d -> p j d", p=P)   # [128, 64, 64]
            yv = y.rearrange("(j p) d -> p j d", p=P)
            prev_et = None
            for c in range(CHUNKS_PER_SEQ):
                jlo = c * CHUNK_TILES
                xt = x_pool.tile([P, CHUNK_COLS], f32)
                nc.sync.dma_start(
                    xt[:].rearrange("p (j d) -> p j d", d=TILE_D),
                    xv[:, jlo : jlo + CHUNK_TILES, :],
                )
                et_r = e_pool.tile([P, CHUNK_COLS], f32r)
                nc.scalar.activation(et_r[:], xt[:], AF.Exp, scale=-SCALE)

                ps = ps_pool.tile([P, CHUNK_COLS], f32)
                # within-tile band: one weight load, 4 bank-sized matmuls
                for g in range(GROUPS):
                    sl = slice(g * GROUP_COLS, (g + 1) * GROUP_COLS)
                    nc.tensor.matmul(
                        ps[:, sl], w_in_r, et_r[:, sl], start=True, stop=False
                    )
                # previous-tile band (shift one tile right)
                if c == 0:
                    nc.tensor.matmul(
                        ps[:, 0:TILE_D], w_first_r, et_r[:, 0:TILE_D],
                        start=False, stop=False,
                    )
                else:
                    nc.tensor.matmul(
                        ps[:, 0:TILE_D],
                        w_halo_r,
                        prev_et[:, CHUNK_COLS - TILE_D : CHUNK_COLS],
                        start=False, stop=False,
                    )
                nc.tensor.matmul(
                    ps[:, TILE_D:GROUP_COLS],
                    w_halo_r,
                    et_r[:, 0 : GROUP_COLS - TILE_D],
                    start=False, stop=True,
                )
                for g in range(1, GROUPS):
                    nc.tensor.matmul(
                        ps[:, g * GROUP_COLS : (g + 1) * GROUP_COLS],
                        w_halo_r,
                        et_r[:, g * GROUP_COLS - TILE_D : (g + 1) * GROUP_COLS - TILE_D],
                        start=False, stop=True,
                    )

                ot = o_pool.tile([P, CHUNK_COLS], f32)
                nc.scalar.activation(ot[:], ps[:], AF.Ln)
                nc.vector.tensor_scalar_mul(ot[:], ot[:], -1.0 / SCALE)
                nc.sync.dma_start(
                    yv[:, jlo : jlo + CHUNK_TILES, :],
                    ot[:].rearrange("p (j d) -> p j d", d=TILE_D),
                )
                prev_et = et_r
    nc.compile()
    return nc


def _run(lower_trace, upper_trace, trace=False, **spmd_kwargs):
    from concourse.bass_utils import run_bass_kernel_spmd

    lower_trace = np.ascontiguousarray(np.asarray(lower_trace, dtype=np.float32))
    upper_trace = np.ascontiguousarray(np.asarray(upper_trace, dtype=np.float32))
    assert lower_trace.shape == (B, T, D) and upper_trace.shape == (B, T, D)

    nc = _build_bass()
    in_maps = [
        {
            "lower": np.ascontiguousarray(lower_trace[i * B_PER_CORE : (i + 1) * B_PER_CORE]),
            "upper": np.ascontiguousarray(upper_trace[i * B_PER_CORE : (i + 1) * B_PER_CORE]),
        }
        for i in range(N_CORES)
    ]
    res = run_bass_kernel_spmd(
        nc, in_maps, core_ids=list(range(N_CORES)), trace=trace, **spmd_kwargs
    )
    out_lower = np.concatenate([r["out_lower"] for r in res.results], axis=0)
    out_upper = np.concatenate([r["out_upper"] for r in res.results], axis=0)
    return (out_lower, out_upper), res


def kernel(lower_trace, upper_trace):
    (out_lower, out_upper), _ = _run(lower_trace, upper_trace, trace=False)
    return out_lower, out_upper


# revision 11
# speedup vs baseline: 1.1463x; 1.1463x over previous
"""Trainium2 Bass kernel for nn_Always (sliding-window smoothed-min).

The reference "scan" is a sliding-window reduction:
    out[b, t, d] = -(1/5) * log( sum_{k=0..15} exp(-5 * x[b, t-k, d]) )
with x[b, j, d] := x[b, 0, d] for j < 0 (the h0 padding).

Strategy (pure data parallel over 8 cores; 2 batches x 2 tensors per core):
  - layout: time tiles of 256 timesteps: t = 256*J + 2*p + i with p the SBUF
    partition and (i, d) in the free dim. Two consecutive t-rows per
    partition make every DMA descriptor 512 B (vs 256 B for one row),
    which lifts HBM DMA efficiency ~25%.
  - ScalarE: E = exp(-5x)
  - TensorE: banded-matrix matmuls compute the 16-wide window sum S.
    With 2 rows/partition the band splits into 4 (out-parity, in-parity)
    weight pairs per class: W_in[oi][ii] (within-tile), W_halo[oi][ii]
    (previous tile), W_first[oi] (t=0 padding).
  - ScalarE: ln(S) from PSUM
  - VectorE: * -1/5
"""

import numpy as np

B, T, D = 16, 8192, 64
N_CORES = 8
B_PER_CORE = B // N_CORES  # 2
SCALE = 5.0
WIN = 16
P = 128                     # SBUF partitions
ROWS = 2                    # timesteps per partition per tile
TILE_T = P * ROWS           # 256 timesteps per tile
TILE_COLS = ROWS * D        # 128 free columns per tile
CHUNK_TILES = 16            # tiles per chunk
CHUNK_COLS = CHUNK_TILES * TILE_COLS       # 2048
CHUNKS_PER_SEQ = T // (TILE_T * CHUNK_TILES)  # 2
HALF = CHUNK_TILES // 2     # 8 tiles per psum bank


def _ensure_act_root():
    """Point walrus at an act-table root whose set list has
    natural_log_exp_and_others first, so Exp and Ln share one table set
    (otherwise walrus alternates exp_and_others/natural_log, ~2.7us per
    reload on ScalarE)."""
    import json
    import os
    import shutil
    import tempfile

    if os.environ.get("BASS_ACT_ROOT_JSON_PATH"):
        return
    if os.environ.get("DISABLE_ACT_ROOT_FIX"):
        return
    from neuronxcc.driver.Job import Job
    from neuronxcc.driver.jobs.support.FindActInfo import findActInfoFile

    src_json = findActInfoFile(Job.getPackageDir(), "gen3")
    src_dir = os.path.dirname(src_json)
    dst_dir = os.path.join(tempfile.gettempdir(), "act_root_nl_exp_first")
    os.makedirs(dst_dir, exist_ok=True)
    with open(src_json) as f:
        d = json.load(f)
    sets = d["act_func_sets"]
    d["act_func_sets"] = sorted(
        sets, key=lambda s: s["name"] != "natural_log_exp_and_others"
    )
    for fn in os.listdir(src_dir):
        src = os.path.join(src_dir, fn)
        dst = os.path.join(dst_dir, fn)
        if os.path.isfile(src) and not os.path.exists(dst):
            shutil.copy(src, dst)
    with open(os.path.join(dst_dir, "act_info.json"), "w") as f:
        json.dump(d, f)
    os.environ["BASS_ACT_ROOT_JSON_PATH"] = os.path.join(dst_dir, "act_info.json")


def _weight_mats():
    """Returns the 10 banded matrices, concatenated [128, 1280]:
    order: W_in[0][0], W_in[0][1], W_in[1][0], W_in[1][1],
           W_halo[0][0], ..., W_halo[1][1], W_first[0], W_first[1].
    Layout convention: lhsT[p_in, p_out]; matmul computes lhsT.T @ rhs."""
    p = np.arange(P)
    mats = []
    for cls in ("in", "halo"):
        for oi in (0, 1):
            for ii in (0, 1):
                t_out = 2 * p[None, :] + oi
                t_in = 2 * p[:, None] + ii
                dd = t_out - t_in + (TILE_T if cls == "halo" else 0)
                lo = 1 if cls == "halo" else 0
                mats.append(((dd >= lo) & (dd <= WIN - 1)).astype(np.float32))
    for oi in (0, 1):
        wf = np.zeros((P, P), np.float32)
        wf[0, :] = np.maximum(WIN - 1 - (2 * p + oi), 0)
        mats.append(wf)
    return np.concatenate(mats, axis=1)


def _build_bass():
    from contextlib import ExitStack

    import concourse.bacc as bacc
    import concourse.tile as tile
    from concourse import mybir

    f32 = mybir.dt.float32
    f32r = mybir.dt.float32r
    AF = mybir.ActivationFunctionType

    nc = bacc.Bacc(trn_type="TRN2")
    lo = nc.dram_tensor("lower", [B_PER_CORE, T, D], f32, kind="ExternalInput")
    up = nc.dram_tensor("upper", [B_PER_CORE, T, D], f32, kind="ExternalInput")
    out_lo = nc.dram_tensor("out_lower", [B_PER_CORE, T, D], f32, kind="ExternalOutput")
    out_up = nc.dram_tensor("out_upper", [B_PER_CORE, T, D], f32, kind="ExternalOutput")

    w_all_d = nc.inline_tensor(_weight_mats(), name="w_all_c")

    with tile.TileContext(nc) as tc, ExitStack() as ctx:
        consts = ctx.enter_context(tc.tile_pool(name="consts", bufs=1))
        x_pool = ctx.enter_context(tc.tile_pool(name="x", bufs=5))
        e_pool = ctx.enter_context(tc.tile_pool(name="e", bufs=5))
        o_pool = ctx.enter_context(tc.tile_pool(name="o", bufs=5))
        ps_pool = ctx.enter_context(tc.tile_pool(name="ps", bufs=2, space="PSUM"))

        w_all = consts.tile([P, 10 * P], f32r)
        nc.sync.dma_start(w_all[:], w_all_d[:].bitcast(f32r))

        def w(idx):
            return w_all[:, idx * P : (idx + 1) * P]

        W_IN = lambda oi, ii: w(oi * 2 + ii)          # noqa: E731
        W_HALO = lambda oi, ii: w(4 + oi * 2 + ii)    # noqa: E731
        W_FIRST = lambda oi: w(8 + oi)                # noqa: E731

        seqs = []
        for src, dst in ((lo, out_lo), (up, out_up)):
            for b in range(B_PER_CORE):
                seqs.append((src[b], dst[b]))

        for x, y in seqs:
            xv = x.rearrange("(J p i) d -> p J i d", p=P, i=ROWS)  # [128,32,2,64]
            yv = y.rearrange("(J p i) d -> p J i d", p=P, i=ROWS)
            prev_et = None
            for c in range(CHUNKS_PER_SEQ):
                J0 = c * CHUNK_TILES
                xt = x_pool.tile([P, CHUNK_COLS], f32)
                nc.sync.dma_start(
                    xt[:].rearrange("p (J i d) -> p J i d", i=ROWS, d=D),
                    xv[:, J0 : J0 + CHUNK_TILES, :, :],
                )
                et = e_pool.tile([P, CHUNK_COLS], f32r)
                nc.scalar.activation(et[:], xt[:], AF.Exp, scale=-SCALE)
                et3 = et[:].rearrange("p (J i d) -> p J i d", i=ROWS, d=D)
                pe3 = (
                    prev_et[:].rearrange("p (J i d) -> p J i d", i=ROWS, d=D)
                    if prev_et is not None
                    else None
                )

                # psum layout: col = oi*1024 + J*64 + d  (J within chunk)
                ps = ps_pool.tile([P, CHUNK_COLS], f32)

                # (bank_key, lhsT, rhs, out) in emission order; start/stop
                # computed per psum bank (oi, half).
                mms = []

                def out_ap(oi, j_lo, j_hi):
                    return ps[:, oi * 1024 + j_lo * D : oi * 1024 + j_hi * D]

                for oi in (0, 1):
                    for ii in (0, 1):
                        lh = W_IN(oi, ii)
                        for h in (0, 1):
                            mms.append((
                                (oi, h), lh,
                                et3[:, h * HALF : (h + 1) * HALF, ii, :],
                                out_ap(oi, h * HALF, (h + 1) * HALF),
                            ))
                for oi in (0, 1):
                    for ii in (0, 1):
                        lh = W_HALO(oi, ii)
                        if c > 0:
                            mms.append((
                                (oi, 0), lh,
                                pe3[:, CHUNK_TILES - 1 : CHUNK_TILES, ii, :],
                                out_ap(oi, 0, 1),
                            ))
                        mms.append((
                            (oi, 0), lh,
                            et3[:, 0 : HALF - 1, ii, :],
                            out_ap(oi, 1, HALF),
                        ))
                        mms.append((
                            (oi, 1), lh,
                            et3[:, HALF - 1 : CHUNK_TILES - 1, ii, :],
                            out_ap(oi, HALF, CHUNK_TILES),
                        ))
                if c == 0:
                    for oi in (0, 1):
                        mms.append((
                            (oi, 0), W_FIRST(oi),
                            et3[:, 0:1, 0, :],
                            out_ap(oi, 0, 1),
                        ))

                first_seen, last_idx = set(), {}
                for k, (bank, *_rest) in enumerate(mms):
                    last_idx[bank] = k
                for k, (bank, lh, rhs, outp) in enumerate(mms):
                    st = bank not in first_seen
                    first_seen.add(bank)
                    nc.tensor.matmul(
                        outp, lh, rhs, start=st, stop=(last_idx[bank] == k)
                    )

                ot = o_pool.tile([P, CHUNK_COLS], f32)
                # ps iterates (oi, J, d); ot memory layout is (J, i, d)
                ot_perm = ot[:].rearrange("p (J i d) -> p i J d", i=ROWS, d=D)
                nc.scalar.activation(ot_perm, ps[:], AF.Ln)
                nc.vector.tensor_scalar_mul(ot[:], ot[:], -1.0 / SCALE)
                nc.sync.dma_start(
                    yv[:, J0 : J0 + CHUNK_TILES, :, :],
                    ot[:].rearrange("p (J i d) -> p J i d", i=ROWS, d=D),
                )
                prev_et = et
    nc.compile()
    return nc


def _run(lower_trace, upper_trace, trace=False, **spmd_kwargs):
    from concourse.bass_utils import run_bass_kernel_spmd

    _ensure_act_root()

    lower_trace = np.ascontiguousarray(np.asarray(lower_trace, dtype=np.float32))
    upper_trace = np.ascontiguousarray(np.asarray(upper_trace, dtype=np.float32))
    assert lower_trace.shape == (B, T, D) and upper_trace.shape == (B, T, D)

    nc = _build_bass()
    in_maps = [
        {
            "lower": np.ascontiguousarray(lower_trace[i * B_PER_CORE : (i + 1) * B_PER_CORE]),
            "upper": np.ascontiguousarray(upper_trace[i * B_PER_CORE : (i + 1) * B_PER_CORE]),
        }
        for i in range(N_CORES)
    ]
    res = run_bass_kernel_spmd(
        nc, in_maps, core_ids=list(range(N_CORES)), trace=trace, **spmd_kwargs
    )
    out_lower = np.concatenate([r["out_lower"] for r in res.results], axis=0)
    out_upper = np.concatenate([r["out_upper"] for r in res.results], axis=0)
    return (out_lower, out_upper), res


def kernel(lower_trace, upper_trace):
    (out_lower, out_upper), _ = _run(lower_trace, upper_trace, trace=False)
    return out_lower, out_upper


# revision 14
# speedup vs baseline: 1.1575x; 1.0097x over previous
"""Trainium2 Bass kernel for nn_Always (sliding-window smoothed-min).

The reference "scan" is a sliding-window reduction:
    out[b, t, d] = -(1/5) * log( sum_{k=0..15} exp(-5 * x[b, t-k, d]) )
with x[b, j, d] := x[b, 0, d] for j < 0 (the h0 padding).

Strategy (pure data parallel over 8 cores; 2 batches x 2 tensors per core):
  - layout: time tiles of 256 timesteps: t = 256*J + 2*p + i with p the SBUF
    partition and (i, d) in the free dim. Two consecutive t-rows per
    partition make every DMA descriptor 512 B, lifting HBM DMA efficiency.
  - ScalarE: E = exp(-5x), in place (f32r tiles)
  - TensorE: banded-matrix matmuls compute the 16-wide window sum S.
    With 2 rows/partition the band splits into 4 (out-parity, in-parity)
    weight pairs per class: W_in[oi][ii] (within-tile), W_halo[oi][ii]
    (previous tile), W_first[oi] (t=0 padding).
  - ScalarE: ln(S) from PSUM; VectorE: * -1/5
  Scheduling structure: all 8 input DMAs are emitted first on the SP
  sequencer (so no input transfer ever queues behind an output DMA whose
  semaphore wait is still pending), activations run in groups of 4 exps /
  4 lns (minimizes ACT table-set reloads between Exp and Ln), and all
  output DMAs are emitted last.
"""

import numpy as np

B, T, D = 16, 8192, 64
N_CORES = 8
B_PER_CORE = B // N_CORES  # 2
SCALE = 5.0
WIN = 16
P = 128                     # SBUF partitions
ROWS = 2                    # timesteps per partition per tile
TILE_T = P * ROWS           # 256 timesteps per tile
TILE_COLS = ROWS * D        # 128 free columns per tile
CHUNK_TILES = 16            # tiles per chunk
CHUNK_COLS = CHUNK_TILES * TILE_COLS       # 2048
CHUNKS_PER_SEQ = T // (TILE_T * CHUNK_TILES)  # 2
HALF = CHUNK_TILES // 2     # 8 tiles per psum bank
N_SEQS = 2 * B_PER_CORE     # 4 sequences per core
N_CHUNKS = N_SEQS * CHUNKS_PER_SEQ  # 8
ACT_GROUP = 4               # chunks per exp/ln activation group


def _weight_mats():
    """Returns the 10 banded matrices, concatenated [128, 1280]:
    order: W_in[0][0], W_in[0][1], W_in[1][0], W_in[1][1],
           W_halo[0][0], ..., W_halo[1][1], W_first[0], W_first[1].
    Layout convention: lhsT[p_in, p_out]; matmul computes lhsT.T @ rhs."""
    p = np.arange(P)
    mats = []
    for cls in ("in", "halo"):
        for oi in (0, 1):
            for ii in (0, 1):
                t_out = 2 * p[None, :] + oi
                t_in = 2 * p[:, None] + ii
                dd = t_out - t_in + (TILE_T if cls == "halo" else 0)
                lo = 1 if cls == "halo" else 0
                mats.append(((dd >= lo) & (dd <= WIN - 1)).astype(np.float32))
    for oi in (0, 1):
        wf = np.zeros((P, P), np.float32)
        wf[0, :] = np.maximum(WIN - 1 - (2 * p + oi), 0)
        mats.append(wf)
    return np.concatenate(mats, axis=1)


def _build_bass(mode="grouped"):
    from contextlib import ExitStack

    import concourse.bacc as bacc
    import concourse.tile as tile
    from concourse import mybir

    f32 = mybir.dt.float32
    f32r = mybir.dt.float32r
    AF = mybir.ActivationFunctionType

    nc = bacc.Bacc(trn_type="TRN2")
    lo = nc.dram_tensor("lower", [B_PER_CORE, T, D], f32, kind="ExternalInput")
    up = nc.dram_tensor("upper", [B_PER_CORE, T, D], f32, kind="ExternalInput")
    out_lo = nc.dram_tensor("out_lower", [B_PER_CORE, T, D], f32, kind="ExternalOutput")
    out_up = nc.dram_tensor("out_upper", [B_PER_CORE, T, D], f32, kind="ExternalOutput")

    w_all_d = nc.inline_tensor(_weight_mats(), name="w_all_c")

    def view3(ap):
        return ap.rearrange("p (J i d) -> p J i d", i=ROWS, d=D)

    with tile.TileContext(nc) as tc, ExitStack() as ctx:
        consts = ctx.enter_context(tc.tile_pool(name="consts", bufs=1))
        xe_pool = ctx.enter_context(tc.tile_pool(name="xe", bufs=N_CHUNKS))
        o_pool = ctx.enter_context(tc.tile_pool(name="o", bufs=N_CHUNKS))
        ps_pool = ctx.enter_context(tc.tile_pool(name="ps", bufs=2, space="PSUM"))

        w_all = consts.tile([P, 10 * P], f32r)
        nc.sync.dma_start(w_all[:], w_all_d[:].bitcast(f32r))

        def w(idx):
            return w_all[:, idx * P : (idx + 1) * P]

        W_IN = lambda oi, ii: w(oi * 2 + ii)          # noqa: E731
        W_HALO = lambda oi, ii: w(4 + oi * 2 + ii)    # noqa: E731
        W_FIRST = lambda oi: w(8 + oi)                # noqa: E731

        # chunk list: (dram_x_view, dram_y_view, chunk_idx_within_seq)
        chunks = []
        for src, dst in ((lo, out_lo), (up, out_up)):
            for b in range(B_PER_CORE):
                xv = src[b].rearrange("(J p i) d -> p J i d", p=P, i=ROWS)
                yv = dst[b].rearrange("(J p i) d -> p J i d", p=P, i=ROWS)
                for c in range(CHUNKS_PER_SEQ):
                    chunks.append((xv, yv, c))

        def emit_in(q):
            xv, _yv, c = chunks[q]
            J0 = c * CHUNK_TILES
            xt = xe_pool.tile([P, CHUNK_COLS], f32r)
            nc.sync.dma_start(
                view3(xt[:]), xv[:, J0 : J0 + CHUNK_TILES, :, :].bitcast(f32r)
            )
            xts.append(xt)

        xts = []
        if mode == "grouped":
            for q in range(N_CHUNKS):
                emit_in(q)

        # ---- phase B: compute, activation-grouped
        pss = [None] * N_CHUNKS
        ots = [None] * N_CHUNKS

        def emit_mms(q):
            _xv, _yv, c = chunks[q]
            et3 = view3(xts[q][:])
            pe3 = view3(xts[q - 1][:]) if c > 0 else None
            ps = ps_pool.tile([P, CHUNK_COLS], f32)
            pss[q] = ps

            mms = []

            def out_ap(oi, j_lo, j_hi):
                return ps[:, oi * 1024 + j_lo * D : oi * 1024 + j_hi * D]

            for oi in (0, 1):
                for ii in (0, 1):
                    lh = W_IN(oi, ii)
                    for h in (0, 1):
                        mms.append((
                            (oi, h), lh,
                            et3[:, h * HALF : (h + 1) * HALF, ii, :],
                            out_ap(oi, h * HALF, (h + 1) * HALF),
                        ))
            for oi in (0, 1):
                for ii in (0, 1):
                    lh = W_HALO(oi, ii)
                    if c > 0:
                        mms.append((
                            (oi, 0), lh,
                            pe3[:, CHUNK_TILES - 1 : CHUNK_TILES, ii, :],
                            out_ap(oi, 0, 1),
                        ))
                    mms.append((
                        (oi, 0), lh,
                        et3[:, 0 : HALF - 1, ii, :],
                        out_ap(oi, 1, HALF),
                    ))
                    mms.append((
                        (oi, 1), lh,
                        et3[:, HALF - 1 : CHUNK_TILES - 1, ii, :],
                        out_ap(oi, HALF, CHUNK_TILES),
                    ))
            if c == 0:
                for oi in (0, 1):
                    mms.append((
                        (oi, 0), W_FIRST(oi),
                        et3[:, 0:1, 0, :],
                        out_ap(oi, 0, 1),
                    ))

            first_seen, last_idx = set(), {}
            for k, (bank, *_rest) in enumerate(mms):
                last_idx[bank] = k
            for k, (bank, lh, rhs, outp) in enumerate(mms):
                st = bank not in first_seen
                first_seen.add(bank)
                nc.tensor.matmul(outp, lh, rhs, start=st, stop=(last_idx[bank] == k))

        def emit_ln(q):
            ot = o_pool.tile([P, CHUNK_COLS], f32)
            ots[q] = ot
            # ps iterates (oi, J, d); ot memory layout is (J, i, d)
            nc.scalar.activation(
                ot[:].rearrange("p (J i d) -> p i J d", i=ROWS, d=D),
                pss[q][:],
                AF.Ln,
            )
            nc.vector.tensor_scalar_mul(ot[:], ot[:], -1.0 / SCALE)

        def emit_out(q):
            _xv, yv, c = chunks[q]
            J0 = c * CHUNK_TILES
            nc.sync.dma_start(
                yv[:, J0 : J0 + CHUNK_TILES, :, :], view3(ots[q][:])
            )

        if mode == "grouped":
            for g in range(0, N_CHUNKS, ACT_GROUP):
                grp = range(g, min(g + ACT_GROUP, N_CHUNKS))
                for q in grp:
                    nc.scalar.activation(xts[q][:], xts[q][:], AF.Exp, scale=-SCALE)
                for q in grp:
                    emit_mms(q)
                for q in grp:
                    emit_ln(q)
            for q in range(N_CHUNKS):
                emit_out(q)
        elif mode == "perchunk":
            for q in range(N_CHUNKS):
                emit_in(q)
                nc.scalar.activation(xts[q][:], xts[q][:], AF.Exp, scale=-SCALE)
                emit_mms(q)
                emit_ln(q)
                emit_out(q)
        elif mode == "insfirst":
            for q in range(N_CHUNKS):
                emit_in(q)
            for q in range(N_CHUNKS):
                nc.scalar.activation(xts[q][:], xts[q][:], AF.Exp, scale=-SCALE)
                emit_mms(q)
                emit_ln(q)
            for q in range(N_CHUNKS):
                emit_out(q)
        else:
            raise ValueError(mode)
    nc.compile()
    return nc


def _run(lower_trace, upper_trace, trace=False, mode="grouped", **spmd_kwargs):
    from concourse.bass_utils import run_bass_kernel_spmd

    lower_trace = np.ascontiguousarray(np.asarray(lower_trace, dtype=np.float32))
    upper_trace = np.ascontiguousarray(np.asarray(upper_trace, dtype=np.float32))
    assert lower_trace.shape == (B, T, D) and upper_trace.shape == (B, T, D)

    nc = _build_bass(mode=mode)
    in_maps = [
        {
            "lower": np.ascontiguousarray(lower_trace[i * B_PER_CORE : (i + 1) * B_PER_CORE]),
            "upper": np.ascontiguousarray(upper_trace[i * B_PER_CORE : (i + 1) * B_PER_CORE]),
        }
        for i in range(N_CORES)
    ]
    res = run_bass_kernel_spmd(
        nc, in_maps, core_ids=list(range(N_CORES)), trace=trace, **spmd_kwargs
    )
    out_lower = np.concatenate([r["out_lower"] for r in res.results], axis=0)
    out_upper = np.concatenate([r["out_upper"] for r in res.results], axis=0)
    return (out_lower, out_upper), res


def kernel(lower_trace, upper_trace):
    (out_lower, out_upper), _ = _run(lower_trace, upper_trace, trace=False)
    return out_lower, out_upper


# revision 17
# speedup vs baseline: 1.1614x; 1.0034x over previous
"""Trainium2 Bass kernel for nn_Always (sliding-window smoothed-min).

The reference "scan" is a sliding-window reduction:
    out[b, t, d] = -(1/5) * log( sum_{k=0..15} exp(-5 * x[b, t-k, d]) )
with x[b, j, d] := x[b, 0, d] for j < 0 (the h0 padding).

Strategy (pure data parallel over 8 cores; 2 batches x 2 tensors per core):
  - layout: time tiles of 256 timesteps: t = 256*J + 2*p + i with p the SBUF
    partition and (i, d) in the free dim. Two consecutive t-rows per
    partition make every DMA descriptor 512 B, lifting HBM DMA efficiency.
  - ScalarE: E = exp(-5x), in place (f32r tiles)
  - TensorE: banded-matrix matmuls compute the 16-wide window sum S.
    With 2 rows/partition the band splits into 4 (out-parity, in-parity)
    weight pairs per class: W_in[oi][ii] (within-tile), W_halo[oi][ii]
    (previous tile), W_first[oi] (t=0 padding).
  - ScalarE: ln(S) from PSUM; VectorE: * -1/5
  Scheduling structure: all 8 input DMAs are emitted first on the SP
  sequencer (so no input transfer ever queues behind an output DMA whose
  semaphore wait is still pending), activations run in groups of 4 exps /
  4 lns (minimizes ACT table-set reloads between Exp and Ln), and all
  output DMAs are emitted last.
"""

import numpy as np

B, T, D = 16, 8192, 64
N_CORES = 8
B_PER_CORE = B // N_CORES  # 2
SCALE = 5.0
WIN = 16
P = 128                     # SBUF partitions
ROWS = 2                    # timesteps per partition per tile
TILE_T = P * ROWS           # 256 timesteps per tile
TILE_COLS = ROWS * D        # 128 free columns per tile
CHUNK_TILES = 16            # tiles per chunk
CHUNK_COLS = CHUNK_TILES * TILE_COLS       # 2048
CHUNKS_PER_SEQ = T // (TILE_T * CHUNK_TILES)  # 2
HALF = CHUNK_TILES // 2     # 8 tiles per psum bank
N_SEQS = 2 * B_PER_CORE     # 4 sequences per core
N_CHUNKS = N_SEQS * CHUNKS_PER_SEQ  # 8
ACT_GROUP = 4               # chunks per exp/ln activation group


def _weight_mats():
    """Returns the 10 banded matrices, concatenated [128, 1280]:
    order: W_in[0][0], W_in[0][1], W_in[1][0], W_in[1][1],
           W_halo[0][0], ..., W_halo[1][1], W_first[0], W_first[1].
    Layout convention: lhsT[p_in, p_out]; matmul computes lhsT.T @ rhs."""
    p = np.arange(P)
    mats = []
    for cls in ("in", "halo"):
        for oi in (0, 1):
            for ii in (0, 1):
                t_out = 2 * p[None, :] + oi
                t_in = 2 * p[:, None] + ii
                dd = t_out - t_in + (TILE_T if cls == "halo" else 0)
                lo = 1 if cls == "halo" else 0
                mats.append(((dd >= lo) & (dd <= WIN - 1)).astype(np.float32))
    for oi in (0, 1):
        wf = np.zeros((P, P), np.float32)
        wf[0, :] = np.maximum(WIN - 1 - (2 * p + oi), 0)
        mats.append(wf)
    return np.concatenate(mats, axis=1)


def _build_bass(mode="grouped"):
    from contextlib import ExitStack

    import concourse.bacc as bacc
    import concourse.tile as tile
    from concourse import mybir

    f32 = mybir.dt.float32
    f32r = mybir.dt.float32r
    AF = mybir.ActivationFunctionType

    nc = bacc.Bacc(trn_type="TRN2")
    lo = nc.dram_tensor("lower", [B_PER_CORE, T, D], f32, kind="ExternalInput")
    up = nc.dram_tensor("upper", [B_PER_CORE, T, D], f32, kind="ExternalInput")
    out_lo = nc.dram_tensor("out_lower", [B_PER_CORE, T, D], f32, kind="ExternalOutput")
    out_up = nc.dram_tensor("out_upper", [B_PER_CORE, T, D], f32, kind="ExternalOutput")

    w_all_d = nc.inline_tensor(_weight_mats(), name="w_all_c")

    def view3(ap):
        return ap.rearrange("p (J i d) -> p J i d", i=ROWS, d=D)

    with tile.TileContext(nc) as tc, ExitStack() as ctx:
        consts = ctx.enter_context(tc.tile_pool(name="consts", bufs=1))
        xe_pool = ctx.enter_context(tc.tile_pool(name="xe", bufs=N_CHUNKS))
        o_pool = ctx.enter_context(tc.tile_pool(name="o", bufs=N_CHUNKS))
        ps_pool = ctx.enter_context(tc.tile_pool(name="ps", bufs=2, space="PSUM"))

        w_all = consts.tile([P, 10 * P], f32r)
        nc.sync.dma_start(w_all[:], w_all_d[:].bitcast(f32r))

        def w(idx):
            return w_all[:, idx * P : (idx + 1) * P]

        W_IN = lambda oi, ii: w(oi * 2 + ii)          # noqa: E731
        W_HALO = lambda oi, ii: w(4 + oi * 2 + ii)    # noqa: E731
        W_FIRST = lambda oi: w(8 + oi)                # noqa: E731

        # chunk list: (dram_x_view, dram_y_view, chunk_idx_within_seq)
        chunks = []
        for src, dst in ((lo, out_lo), (up, out_up)):
            for b in range(B_PER_CORE):
                xv = src[b].rearrange("(J p i) d -> p J i d", p=P, i=ROWS)
                yv = dst[b].rearrange("(J p i) d -> p J i d", p=P, i=ROWS)
                for c in range(CHUNKS_PER_SEQ):
                    chunks.append((xv, yv, c))

        def emit_in(q):
            xv, _yv, c = chunks[q]
            J0 = c * CHUNK_TILES
            xt = xe_pool.tile([P, CHUNK_COLS], f32r)
            nc.sync.dma_start(
                view3(xt[:]), xv[:, J0 : J0 + CHUNK_TILES, :, :].bitcast(f32r)
            )
            xts.append(xt)

        xts = []
        if mode == "grouped":
            for q in range(N_CHUNKS):
                emit_in(q)

        # ---- phase B: compute, activation-grouped
        pss = [None] * N_CHUNKS
        ots = [None] * N_CHUNKS

        def emit_mms(q):
            _xv, _yv, c = chunks[q]
            et3 = view3(xts[q][:])
            pe3 = view3(xts[q - 1][:]) if c > 0 else None
            ps = ps_pool.tile([P, CHUNK_COLS], f32)
            pss[q] = ps

            mms = []

            def out_ap(oi, j_lo, j_hi):
                return ps[:, oi * 1024 + j_lo * D : oi * 1024 + j_hi * D]

            for oi in (0, 1):
                for ii in (0, 1):
                    lh = W_IN(oi, ii)
                    for h in (0, 1):
                        mms.append((
                            (oi, h), lh,
                            et3[:, h * HALF : (h + 1) * HALF, ii, :],
                            out_ap(oi, h * HALF, (h + 1) * HALF),
                        ))
            for oi in (0, 1):
                for ii in (0, 1):
                    lh = W_HALO(oi, ii)
                    if c > 0:
                        mms.append((
                            (oi, 0), lh,
                            pe3[:, CHUNK_TILES - 1 : CHUNK_TILES, ii, :],
                            out_ap(oi, 0, 1),
                        ))
                    mms.append((
                        (oi, 0), lh,
                        et3[:, 0 : HALF - 1, ii, :],
                        out_ap(oi, 1, HALF),
                    ))
                    mms.append((
                        (oi, 1), lh,
                        et3[:, HALF - 1 : CHUNK_TILES - 1, ii, :],
                        out_ap(oi, HALF, CHUNK_TILES),
                    ))
            if c == 0:
                for oi in (0, 1):
                    mms.append((
                        (oi, 0), W_FIRST(oi),
                        et3[:, 0:1, 0, :],
                        out_ap(oi, 0, 1),
                    ))

            first_seen, last_idx = set(), {}
            for k, (bank, *_rest) in enumerate(mms):
                last_idx[bank] = k
            for k, (bank, lh, rhs, outp) in enumerate(mms):
                st = bank not in first_seen
                first_seen.add(bank)
                nc.tensor.matmul(outp, lh, rhs, start=st, stop=(last_idx[bank] == k))

        def emit_ln(q):
            ot = o_pool.tile([P, CHUNK_COLS], f32)
            ots[q] = ot
            # ps iterates (oi, J, d); ot memory layout is (J, i, d)
            nc.scalar.activation(
                ot[:].rearrange("p (J i d) -> p i J d", i=ROWS, d=D),
                pss[q][:],
                AF.Ln,
            )
            nc.vector.tensor_scalar_mul(ot[:], ot[:], -1.0 / SCALE)

        def emit_out(q):
            _xv, yv, c = chunks[q]
            J0 = c * CHUNK_TILES
            nc.sync.dma_start(
                yv[:, J0 : J0 + CHUNK_TILES, :, :], view3(ots[q][:])
            )

        if mode == "grouped":
            for g in range(0, N_CHUNKS, ACT_GROUP):
                grp = range(g, min(g + ACT_GROUP, N_CHUNKS))
                for q in grp:
                    nc.scalar.activation(
                        xts[q][:], xts[q][:].bitcast(f32), AF.Exp, scale=-SCALE
                    )
                for q in grp:
                    emit_mms(q)
                for q in grp:
                    emit_ln(q)
            for q in range(N_CHUNKS):
                emit_out(q)
        elif mode == "perchunk":
            for q in range(N_CHUNKS):
                emit_in(q)
                nc.scalar.activation(
                    xts[q][:], xts[q][:].bitcast(f32), AF.Exp, scale=-SCALE
                )
                emit_mms(q)
                emit_ln(q)
                emit_out(q)
        elif mode == "insfirst":
            for q in range(N_CHUNKS):
                emit_in(q)
            for q in range(N_CHUNKS):
                nc.scalar.activation(
                    xts[q][:], xts[q][:].bitcast(f32), AF.Exp, scale=-SCALE
                )
                emit_mms(q)
                emit_ln(q)
            for q in range(N_CHUNKS):
                emit_out(q)
        else:
            raise ValueError(mode)
    nc.compile()
    return nc


def _run(lower_trace, upper_trace, trace=False, mode="grouped", **spmd_kwargs):
    from concourse.bass_utils import run_bass_kernel_spmd

    lower_trace = np.ascontiguousarray(np.asarray(lower_trace, dtype=np.float32))
    upper_trace = np.ascontiguousarray(np.asarray(upper_trace, dtype=np.float32))
    assert lower_trace.shape == (B, T, D) and upper_trace.shape == (B, T, D)

    nc = _build_bass(mode=mode)
    in_maps = [
        {
            "lower": np.ascontiguousarray(lower_trace[i * B_PER_CORE : (i + 1) * B_PER_CORE]),
            "upper": np.ascontiguousarray(upper_trace[i * B_PER_CORE : (i + 1) * B_PER_CORE]),
        }
        for i in range(N_CORES)
    ]
    res = run_bass_kernel_spmd(
        nc, in_maps, core_ids=list(range(N_CORES)), trace=trace, **spmd_kwargs
    )
    out_lower = np.concatenate([r["out_lower"] for r in res.results], axis=0)
    out_upper = np.concatenate([r["out_upper"] for r in res.results], axis=0)
    return (out_lower, out_upper), res


def kernel(lower_trace, upper_trace):
    (out_lower, out_upper), _ = _run(lower_trace, upper_trace, trace=False)
    return out_lower, out_upper


# revision 18
# speedup vs baseline: 1.3333x; 1.1479x over previous
"""Trainium2 Bass kernel for nn_Always (sliding-window smoothed-min).

The reference "scan" is a sliding-window reduction:
    out[b, t, d] = -(1/5) * log( sum_{k=0..15} exp(-5 * x[b, t-k, d]) )
with x[b, j, d] := x[b, 0, d] for j < 0 (the h0 padding).

Strategy (pure data parallel over 8 cores; 2 batches x 2 tensors per core):
  - layout: time tiles of 256 timesteps: t = 256*J + 2*p + i with p the SBUF
    partition and (i, d) in the free dim. Two consecutive t-rows per
    partition make every DMA descriptor 512 B, lifting HBM DMA efficiency.
  - ScalarE: E = exp(-5x), in place (f32r tiles)
  - TensorE: banded-matrix matmuls compute the 16-wide window sum S.
    With 2 rows/partition the band splits into 4 (out-parity, in-parity)
    weight pairs per class: W_in[oi][ii] (within-tile), W_halo[oi][ii]
    (previous tile), W_first[oi] (t=0 padding).
  - ScalarE: ln(S) from PSUM; VectorE: * -1/5
  Scheduling structure: all 8 input DMAs are emitted first on the SP
  sequencer (so no input transfer ever queues behind an output DMA whose
  semaphore wait is still pending), activations run in groups of 4 exps /
  4 lns (minimizes ACT table-set reloads between Exp and Ln), and all
  output DMAs are emitted last.
"""

import numpy as np

B, T, D = 16, 8192, 64
N_CORES = 8
B_PER_CORE = B // N_CORES  # 2
SCALE = 5.0
WIN = 16
P = 128                     # SBUF partitions
ROWS = 2                    # timesteps per partition per tile
TILE_T = P * ROWS           # 256 timesteps per tile
TILE_COLS = ROWS * D        # 128 free columns per tile
CHUNK_TILES = 16            # tiles per chunk
CHUNK_COLS = CHUNK_TILES * TILE_COLS       # 2048
CHUNKS_PER_SEQ = T // (TILE_T * CHUNK_TILES)  # 2
HALF = CHUNK_TILES // 2     # 8 tiles per psum bank
N_SEQS = 2 * B_PER_CORE     # 4 sequences per core
N_CHUNKS = N_SEQS * CHUNKS_PER_SEQ  # 8
ACT_GROUP = 4               # chunks per exp/ln activation group


def _weight_mats():
    """Returns the 10 banded matrices, concatenated [128, 1280]:
    order: W_in[0][0], W_in[0][1], W_in[1][0], W_in[1][1],
           W_halo[0][0], ..., W_halo[1][1], W_first[0], W_first[1].
    Layout convention: lhsT[p_in, p_out]; matmul computes lhsT.T @ rhs."""
    p = np.arange(P)
    mats = []
    for cls in ("in", "halo"):
        for oi in (0, 1):
            for ii in (0, 1):
                t_out = 2 * p[None, :] + oi
                t_in = 2 * p[:, None] + ii
                dd = t_out - t_in + (TILE_T if cls == "halo" else 0)
                lo = 1 if cls == "halo" else 0
                mats.append(((dd >= lo) & (dd <= WIN - 1)).astype(np.float32))
    for oi in (0, 1):
        wf = np.zeros((P, P), np.float32)
        wf[0, :] = np.maximum(WIN - 1 - (2 * p + oi), 0)
        mats.append(wf)
    return np.concatenate(mats, axis=1)


def _build_bass(mode="grouped"):
    from contextlib import ExitStack

    import concourse.bacc as bacc
    import concourse.tile as tile
    from concourse import mybir

    f32 = mybir.dt.float32
    f32r = mybir.dt.float32r
    AF = mybir.ActivationFunctionType

    nc = bacc.Bacc(trn_type="TRN2")
    lo = nc.dram_tensor("lower", [B_PER_CORE, T, D], f32, kind="ExternalInput")
    up = nc.dram_tensor("upper", [B_PER_CORE, T, D], f32, kind="ExternalInput")
    out_lo = nc.dram_tensor("out_lower", [B_PER_CORE, T, D], f32, kind="ExternalOutput")
    out_up = nc.dram_tensor("out_upper", [B_PER_CORE, T, D], f32, kind="ExternalOutput")

    w_all_d = nc.inline_tensor(_weight_mats(), name="w_all_c")

    def view3(ap):
        return ap.rearrange("p (J i d) -> p J i d", i=ROWS, d=D)

    with tile.TileContext(nc) as tc, ExitStack() as ctx:
        consts = ctx.enter_context(tc.tile_pool(name="consts", bufs=1))
        x_pool = ctx.enter_context(tc.tile_pool(name="x", bufs=6))
        e_pool = ctx.enter_context(tc.tile_pool(name="e", bufs=N_CHUNKS))
        o_pool = ctx.enter_context(tc.tile_pool(name="o", bufs=4))
        ps_pool = ctx.enter_context(tc.tile_pool(name="ps", bufs=2, space="PSUM"))

        w_all = consts.tile([P, 10 * P], f32r)
        nc.sync.dma_start(w_all[:], w_all_d[:].bitcast(f32r))

        def w(idx):
            return w_all[:, idx * P : (idx + 1) * P]

        W_IN = lambda oi, ii: w(oi * 2 + ii)          # noqa: E731
        W_HALO = lambda oi, ii: w(4 + oi * 2 + ii)    # noqa: E731
        W_FIRST = lambda oi: w(8 + oi)                # noqa: E731

        # chunk list: (dram_x_view, dram_y_view, chunk_idx_within_seq)
        chunks = []
        for src, dst in ((lo, out_lo), (up, out_up)):
            for b in range(B_PER_CORE):
                xv = src[b].rearrange("(J p i) d -> p J i d", p=P, i=ROWS)
                yv = dst[b].rearrange("(J p i) d -> p J i d", p=P, i=ROWS)
                for c in range(CHUNKS_PER_SEQ):
                    chunks.append((xv, yv, c))

        def emit_in(q):
            xv, _yv, c = chunks[q]
            J0 = c * CHUNK_TILES
            xt = x_pool.tile([P, CHUNK_COLS], f32)
            nc.sync.dma_start(view3(xt[:]), xv[:, J0 : J0 + CHUNK_TILES, :, :])
            xts.append(xt)

        def emit_exp(q):
            et = e_pool.tile([P, CHUNK_COLS], f32r)
            nc.scalar.activation(et[:], xts[q][:], AF.Exp, scale=-SCALE)
            ets.append(et)

        xts = []
        ets = []
        if mode == "grouped":
            for q in range(N_CHUNKS):
                emit_in(q)

        # ---- phase B: compute, activation-grouped
        pss = [None] * N_CHUNKS
        ots = [None] * N_CHUNKS

        def emit_mms(q):
            _xv, _yv, c = chunks[q]
            et3 = view3(ets[q][:])
            pe3 = view3(ets[q - 1][:]) if c > 0 else None
            ps = ps_pool.tile([P, CHUNK_COLS], f32)
            pss[q] = ps

            mms = []

            def out_ap(oi, j_lo, j_hi):
                return ps[:, oi * 1024 + j_lo * D : oi * 1024 + j_hi * D]

            for oi in (0, 1):
                for ii in (0, 1):
                    lh = W_IN(oi, ii)
                    for h in (0, 1):
                        mms.append((
                            (oi, h), lh,
                            et3[:, h * HALF : (h + 1) * HALF, ii, :],
                            out_ap(oi, h * HALF, (h + 1) * HALF),
                        ))
            for oi in (0, 1):
                for ii in (0, 1):
                    lh = W_HALO(oi, ii)
                    if c > 0:
                        mms.append((
                            (oi, 0), lh,
                            pe3[:, CHUNK_TILES - 1 : CHUNK_TILES, ii, :],
                            out_ap(oi, 0, 1),
                        ))
                    mms.append((
                        (oi, 0), lh,
                        et3[:, 0 : HALF - 1, ii, :],
                        out_ap(oi, 1, HALF),
                    ))
                    mms.append((
                        (oi, 1), lh,
                        et3[:, HALF - 1 : CHUNK_TILES - 1, ii, :],
                        out_ap(oi, HALF, CHUNK_TILES),
                    ))
            if c == 0:
                for oi in (0, 1):
                    mms.append((
                        (oi, 0), W_FIRST(oi),
                        et3[:, 0:1, 0, :],
                        out_ap(oi, 0, 1),
                    ))

            first_seen, last_idx = set(), {}
            for k, (bank, *_rest) in enumerate(mms):
                last_idx[bank] = k
            for k, (bank, lh, rhs, outp) in enumerate(mms):
                st = bank not in first_seen
                first_seen.add(bank)
                nc.tensor.matmul(outp, lh, rhs, start=st, stop=(last_idx[bank] == k))

        def emit_ln(q):
            ot = o_pool.tile([P, CHUNK_COLS], f32)
            ots[q] = ot
            # ps iterates (oi, J, d); ot memory layout is (J, i, d)
            nc.scalar.activation(
                ot[:].rearrange("p (J i d) -> p i J d", i=ROWS, d=D),
                pss[q][:],
                AF.Ln,
            )
            nc.vector.tensor_scalar_mul(ot[:], ot[:], -1.0 / SCALE)

        def emit_out(q):
            _xv, yv, c = chunks[q]
            J0 = c * CHUNK_TILES
            nc.sync.dma_start(
                yv[:, J0 : J0 + CHUNK_TILES, :, :], view3(ots[q][:])
            )

        if mode == "grouped":
            for g in range(0, N_CHUNKS, ACT_GROUP):
                grp = range(g, min(g + ACT_GROUP, N_CHUNKS))
                for q in grp:
                    emit_exp(q)
                for q in grp:
                    emit_mms(q)
                for q in grp:
                    emit_ln(q)
            for q in range(N_CHUNKS):
                emit_out(q)
        elif mode == "perchunk":
            for q in range(N_CHUNKS):
                emit_in(q)
                emit_exp(q)
                emit_mms(q)
                emit_ln(q)
                emit_out(q)
        elif mode == "insfirst":
            for q in range(N_CHUNKS):
                emit_in(q)
            for q in range(N_CHUNKS):
                emit_exp(q)
                emit_mms(q)
                emit_ln(q)
            for q in range(N_CHUNKS):
                emit_out(q)
        else:
            raise ValueError(mode)
    nc.compile()
    return nc


def _run(lower_trace, upper_trace, trace=False, mode="grouped", **spmd_kwargs):
    from concourse.bass_utils import run_bass_kernel_spmd

    lower_trace = np.ascontiguousarray(np.asarray(lower_trace, dtype=np.float32))
    upper_trace = np.ascontiguousarray(np.asarray(upper_trace, dtype=np.float32))
    assert lower_trace.shape == (B, T, D) and upper_trace.shape == (B, T, D)

    nc = _build_bass(mode=mode)
    in_maps = [
        {
            "lower": np.ascontiguousarray(lower_trace[i * B_PER_CORE : (i + 1) * B_PER_CORE]),
            "upper": np.ascontiguousarray(upper_trace[i * B_PER_CORE : (i + 1) * B_PER_CORE]),
        }
        for i in range(N_CORES)
    ]
    res = run_bass_kernel_spmd(
        nc, in_maps, core_ids=list(range(N_CORES)), trace=trace, **spmd_kwargs
    )
    out_lower = np.concatenate([r["out_lower"] for r in res.results], axis=0)
    out_upper = np.concatenate([r["out_upper"] for r in res.results], axis=0)
    return (out_lower, out_upper), res


def kernel(lower_trace, upper_trace):
    (out_lower, out_upper), _ = _run(lower_trace, upper_trace, trace=False)
    return out_lower, out_upper
